# revision 26
# baseline (speedup 1.0000x reference)
"""Multi-head attention (B=4, S=2048, D=1024, H=16) on 8 trn2 NeuronCores.

Sharding: batch (4-way) x head-half (2-way).  Core c = 2*b + hh handles batch b
and heads hh*8 .. hh*8+7.  All matmuls run in bf16 (fp32 PSUM accumulation).

Per core:
  1. QT/KT projections in d-on-partitions layout, V in tokens-on-partitions
     layout with a ones-column per head (softmax denominator comes out of the
     attn@V matmul as row 64).
  2. Attention runs over 512-query blocks and head PAIRS (rows 0:64 / 64:128
     of one dim chunk): the two scores matmuls use disjoint PE row groups and
     overlap in the array.  Per (pair, kt): two scores matmuls, two exps (one
     on the scalar ACT pipe, one as a cubic+square-square polynomial custom op
     on the vector engine), two attn@V matmuls.  Fully double-buffered in
     8 PSUM banks (4 scores + 2 accumulators + 2 output-projection).
  3. Tail per head: denominator row replicated from PSUM with stream
     shuffles, fast reciprocal, multiply into bf16 outT.  Output projection
     for block qb-1 is interleaved into block qb's pair loop to fill PE slack.
     Host sums the two half-head partials and adds the bias.
"""

import sys

if "/opt/trn_rl_repo" not in sys.path:
    sys.path.insert(0, "/opt/trn_rl_repo")

import numpy as np

B, S, D = 4, 2048, 1024
H, HD = 16, 64
P = 128
DK = D // P            # 8 contraction chunks for the projections
NKT = S // P           # 16 token tiles
QB = 512
NQB = S // QB          # 4 q blocks
DH = 512               # head dims per core (8 heads)
NDC = DH // P          # 4 dout chunks per core
NHC = 8                # heads per core
NPAIR = NHC // 2       # head pairs per core
VW = HD + 1            # V columns per head incl. the ones column
NCORES = 8

_PROG = [None]

# exp(s) ~= (1 + s*(EA + s*(EB + s*EC)))**4, minimax on |s| <= 2.3 (scores
# measured in [-1.96, 2.08]); max rel err ~3e-3.
EA, EB, EC = 0.25036017, 0.03195562, 0.00254225


def _register_dve_exp():
    from concourse import dve_ops
    from concourse.dve_spec import Spec, Src0, One, C0, C1, C2, lower
    from concourse.dve_uop import DveOpSpec

    name = "EXP_P3SQ2_ANT"
    for op in dve_ops.OPS:
        if op.name == name:
            return op

    def ref(in0, in1, c0, c1, c2):
        x = np.asarray(in0, np.float32)
        p = (
            np.float32(1.0)
            + x * (np.float32(c0) + x * (np.float32(c1) + x * np.float32(c2)))
        ).astype(np.float32)
        q = (p * p).astype(np.float32)
        return (q * q).astype(np.float32)

    e = ((Src0 * C2 + C1) * Src0 + C0) * Src0
    p = e + One
    q = p * p
    spec = Spec(body=q * q, reference=ref)
    op = dve_ops.DveOp(name, spec, subdim=False, uops_sha={})
    dve_ops.OPS.append(op)
    dve_ops.CUSTOM_DVE_SPECS[name] = spec
    dve_ops._SUB_OPCODE_FOR_NAME[name] = dve_ops._CUSTOM_DVE_ROW_BASE + len(dve_ops.OPS) - 1
    for ver in ("v3", "v4"):
        s = DveOpSpec(
            name=name,
            opcode=dve_ops.get_dve_sub_opcode(name),
            uops=lower(spec, ver=ver),
            rd1_en=False,
        )
        op.uops_sha[ver] = s.sha(ver)
    return op


def _build():
    import concourse.mybir as mybir
    import concourse.tile as tile
    from concourse import bacc

    exp_op = _register_dve_exp()

    f32 = mybir.dt.float32
    bf16 = mybir.dt.bfloat16
    Exp = mybir.ActivationFunctionType.Exp

    nc = bacc.Bacc("TRN2", target_bir_lowering=False, debug=False)
    xq = nc.dram_tensor("xq", [D, S], bf16, kind="ExternalInput").ap()
    xk = nc.dram_tensor("xk", [D, S], bf16, kind="ExternalInput").ap()
    xv = nc.dram_tensor("xv", [D, S], bf16, kind="ExternalInput").ap()
    wq = nc.dram_tensor("wq", [D, DH], bf16, kind="ExternalInput").ap()
    wk = nc.dram_tensor("wk", [D, DH], bf16, kind="ExternalInput").ap()
    wv = nc.dram_tensor("wv", [D, DH], bf16, kind="ExternalInput").ap()
    wo = nc.dram_tensor("wo", [DH, D], bf16, kind="ExternalInput").ap()
    part = nc.dram_tensor("part", [S, D], f32, kind="ExternalOutput").ap()

    xq_v = xq.rearrange("(c p) s -> p c s", p=P)
    xk_v = xk.rearrange("(c p) s -> p c s", p=P)
    xv_v = xv.rearrange("(c p) s -> p c s", p=P)

    with tile.TileContext(nc) as tc:
        with tc.tile_pool(name="big", bufs=1) as big, tc.tile_pool(name="wp", bufs=2) as wp:
            QT = big.tile([P, NDC, S], bf16, tag="QT")
            KT = big.tile([P, NDC, S], bf16, tag="KT")
            V = big.tile([P, NKT, NHC * VW], bf16, tag="V")
            outT = big.tile([P, NDC, S], bf16, tag="outT")

            # preload the exp activation-table set while the first input DMAs
            # are in flight (a mid-kernel table switch costs ~2.7us)
            warm = wp.tile([1, 8], f32, tag="warm")
            nc.vector.memset(warm[:], 0.0)
            nc.scalar.activation(warm[:], warm[:], mybir.ActivationFunctionType.Exp)

            # ---- projections ------------------------------------------------
            with (
                tc.tile_pool(name="xc", bufs=12) as xc,
                tc.tile_pool(name="pp", bufs=4, space="PSUM") as pp,
            ):

                def proj_T(x_view, w_dram, out_t):
                    w_t = wp.tile([P, DK, DH], bf16, tag="w")
                    w_v = w_dram.rearrange("(c p) m -> p c m", p=P)
                    for dk in range(DK):
                        nc.sync.dma_start(w_t[:, dk], w_v[:, dk])
                    for qb in range(NQB):
                        xts = []
                        for dk in range(DK):
                            xt = xc.tile([P, QB], bf16, tag="xc")
                            nc.sync.dma_start(
                                xt[:], x_view[:, dk, qb * QB : (qb + 1) * QB]
                            )
                            xts.append(xt)
                        pts = [pp.tile([P, QB], f32, tag="pp", name=f"pp{i}") for i in range(NDC)]
                        for dk in range(DK):
                            for dc in range(NDC):
                                nc.tensor.matmul(
                                    pts[dc][:],
                                    w_t[:, dk, dc * P : (dc + 1) * P],
                                    xts[dk][:],
                                    start=(dk == 0),
                                    stop=(dk == DK - 1),
                                )
                        for dc in range(NDC):
                            dst = out_t[:, dc, qb * QB : (qb + 1) * QB]
                            if dc % 2 == 0:
                                nc.vector.tensor_copy(dst, pts[dc][:])
                            else:
                                nc.scalar.copy(dst, pts[dc][:])

                proj_T(xq_v, wq, QT)
                proj_T(xk_v, wk, KT)

                # V projection (tokens-on-partitions) + ones columns
                nc.vector.memset(V[:], 1.0)
                wv_t = wp.tile([P, DK, DH], bf16, tag="w")
                wv_v = wv.rearrange("(c p) m -> p c m", p=P)
                for dk in range(DK):
                    nc.sync.dma_start(wv_t[:, dk], wv_v[:, dk])
                for qb in range(NQB):
                    xts = []
                    for dk in range(DK):
                        xt = xc.tile([P, QB], bf16, tag="xc")
                        nc.sync.dma_start(
                            xt[:], xv_v[:, dk, qb * QB : (qb + 1) * QB]
                        )
                        xts.append(xt)
                    pts = [pp.tile([P, DH], f32, tag="pp", name=f"ppv{i}") for i in range(QB // P)]
                    for dk in range(DK):
                        for kt_in in range(QB // P):
                            nc.tensor.matmul(
                                pts[kt_in][:],
                                xts[dk][:, kt_in * P : (kt_in + 1) * P],
                                wv_t[:, dk, :],
                                start=(dk == 0),
                                stop=(dk == DK - 1),
                            )
                    for kt_in in range(QB // P):
                        kt = qb * (QB // P) + kt_in
                        nc.vector.tensor_copy(
                            V[:, kt].rearrange("p (h c) -> p h c", c=VW)[
                                :, :, 0:HD
                            ],
                            pts[kt_in][:].rearrange("p (h c) -> p h c", c=HD),
                        )

            # wo loads while attention runs (slot frees when wv_t is done)
            wo_t = wp.tile([P, NDC, D], bf16, tag="w")
            nc.sync.dma_start(wo_t[:], wo.rearrange("(c p) m -> p c m", p=P))

            # ---- attention + output projection, per 512-query block ---------
            with (
                tc.tile_pool(name="attn", bufs=6) as attnp,
                tc.tile_pool(name="tail", bufs=4) as tailp,
                tc.tile_pool(name="asbp", bufs=4) as asbp,
                tc.tile_pool(name="stage", bufs=2) as stage,
                tc.tile_pool(name="acc", bufs=2, space="PSUM") as accp,
                tc.tile_pool(name="sc", bufs=2, space="PSUM") as scp,
                tc.tile_pool(name="po", bufs=2, space="PSUM") as pop,
            ):

                def emit_oproj_unit(qt, do):
                    po = pop.tile([P, QB], f32, tag="po")
                    for dc in range(NDC):
                        nc.tensor.matmul(
                            po[:],
                            outT[:, dc, qt * P : (qt + 1) * P],
                            wo_t[:, dc, do * QB : (do + 1) * QB],
                            start=(dc == 0),
                            stop=(dc == NDC - 1),
                        )
                    st = stage.tile([P, QB], f32, tag="st")
                    if do == 0:
                        nc.vector.tensor_copy(st[:], po[:])
                    else:
                        nc.scalar.copy(st[:], po[:])
                    # two half-DMAs land on different queues
                    half = QB // 2
                    for j in range(2):
                        nc.sync.dma_start(
                            part[
                                qt * P : (qt + 1) * P,
                                do * QB + j * half : do * QB + (j + 1) * half,
                            ],
                            st[:, j * half : (j + 1) * half],
                        )

                def emit_score_pair_tile(kt, hc, c0):
                    # both heads of the pair into one [P, 2*QB] psum tile
                    # (bank-aligned halves); the two matmuls use disjoint PE
                    # row groups and overlap in the array
                    sct = scp.tile([P, 2 * QB], f32, tag="sc")
                    for hp in range(2):
                        r0 = 64 * hp
                        nc.tensor.matmul(
                            sct[:, hp * QB : (hp + 1) * QB],
                            KT[r0 : r0 + 64, hc, kt * P : (kt + 1) * P],
                            QT[r0 : r0 + 64, hc, c0 : c0 + QB],
                            start=True,
                            stop=True,
                        )
                    return sct

                def emit_exp(at_t, sct, on_act):
                    if on_act:
                        nc.scalar.activation(at_t[:], sct[:], Exp)
                    else:
                        nc.vector._custom_dve(
                            exp_op,
                            out=at_t[:],
                            in0=sct[:],
                            s0=EA,
                            s1=EB,
                            imm2=EC,
                        )

                def emit_attnv(acc, h, kt, at_t):
                    nc.tensor.matmul(
                        acc[0:VW, :],
                        V[:, kt, h * VW : (h + 1) * VW],
                        at_t[:],
                        start=(kt == 0),
                        stop=(kt == NKT - 1),
                    )

                def emit_tail(asb, hp, hc, c0):
                    # asb holds rows 0..64 of the accumulator (evacuated on
                    # the scalar engine): replicate the denominator row, fast
                    # reciprocal, divide into bf16 outT
                    bc = tailp.tile([64, QB], f32, tag="bc")
                    nc.vector.stream_shuffle(bc[0:32, :], asb[64:96, :], [0] * 32)
                    nc.vector.stream_shuffle(bc[32:64, :], asb[64:96, :], [0] * 32)
                    rec = tailp.tile([64, QB], f32, tag="rec")
                    nc.vector.reciprocal_approx_fast(rec[:], bc[:])
                    if hp == 0:
                        nc.vector.tensor_mul(
                            outT[0:64, hc, c0 : c0 + QB], asb[0:HD, :], rec[:]
                        )
                    else:
                        tmp = tailp.tile([64, QB], bf16, tag="tmp")
                        nc.vector.tensor_mul(tmp[:], asb[0:HD, :], rec[:])
                        nc.sync.dma_start(outT[64:128, hc, c0 : c0 + QB], tmp[:])

                # flat software pipeline over (qb, pair, kt): scores run one
                # step ahead, attn@V LAG steps behind, crossing pair
                # boundaries without drain/refill bubbles
                LAG = 2
                steps = [
                    (qb, pair, kt)
                    for qb in range(NQB)
                    for pair in range(NPAIR)
                    for kt in range(NKT)
                ]
                NS = len(steps)
                pending_tail = [None]

                def flush_tail():
                    if pending_tail[0] is not None:
                        a0, a1, phc, pc0 = pending_tail[0]
                        emit_tail(a0, 0, phc, pc0)
                        emit_tail(a1, 1, phc, pc0)
                        pending_tail[0] = None

                accs = {}
                scs = {}
                ats = {}

                def get_accs(qb, pair):
                    if (qb, pair) not in accs:
                        accs[(qb, pair)] = (
                            accp.tile([P, QB], f32, tag="acc", name="acc0"),
                            accp.tile([P, QB], f32, tag="acc", name="acc1"),
                        )
                    return accs[(qb, pair)]

                def emit_score_pair(g):
                    qb, pair, kt = steps[g]
                    scs[g] = emit_score_pair_tile(kt, pair, qb * QB)

                emit_score_pair(0)
                for g in range(NS):
                    qb, pair, kt = steps[g]
                    scq = scs.pop(g)
                    atq = attnp.tile([P, 2 * QB], bf16, tag="attn")
                    ats[g] = atq
                    emit_exp(atq, scq, on_act=(g % 2 == 0))
                    # attn@V (LAG steps behind) first, so the next scores'
                    # LDWEIGHTS can hide under these matmuls
                    if g >= LAG:
                        pqb, ppair, pkt = steps[g - LAG]
                        patq = ats.pop(g - LAG)
                        pa0, pa1 = get_accs(pqb, ppair)
                        emit_attnv(pa0, 2 * ppair, pkt, patq[:, 0:QB])
                        emit_attnv(pa1, 2 * ppair + 1, pkt, patq[:, QB : 2 * QB])
                        if pkt == NKT - 1:
                            # pair finished: evacuate accumulators on the
                            # scalar pipe so the acc banks recycle quickly
                            acc0, acc1 = accs.pop((pqb, ppair))
                            asb0 = asbp.tile([96, QB], f32, tag="asb", name="asb0")
                            asb1 = asbp.tile([96, QB], f32, tag="asb", name="asb1")
                            nc.scalar.copy(asb0[0:VW, :], acc0[0:VW, :])
                            nc.scalar.copy(asb1[0:VW, :], acc1[0:VW, :])
                            flush_tail()
                            pending_tail[0] = (asb0, asb1, ppair, pqb * QB)
                    if g + 1 < NS:
                        emit_score_pair(g + 1)
                    if kt == 6:
                        # previous pair's softmax tail, mid-loop so it fills
                        # vector-engine slack
                        flush_tail()
                    if qb > 0 and kt in (9, 13):
                        # previous block's output projection
                        qt = 4 * (qb - 1) + pair
                        emit_oproj_unit(qt, 0 if kt == 9 else 1)
                # drain the last LAG steps
                for g in range(NS - LAG, NS):
                    pqb, ppair, pkt = steps[g]
                    patq = ats.pop(g)
                    pa0, pa1 = get_accs(pqb, ppair)
                    emit_attnv(pa0, 2 * ppair, pkt, patq[:, 0:QB])
                    emit_attnv(pa1, 2 * ppair + 1, pkt, patq[:, QB : 2 * QB])
                acc0, acc1 = accs.pop((3, NPAIR - 1))
                asb0 = asbp.tile([96, QB], f32, tag="asb", name="asb0")
                asb1 = asbp.tile([96, QB], f32, tag="asb", name="asb1")
                nc.scalar.copy(asb0[0:VW, :], acc0[0:VW, :])
                nc.scalar.copy(asb1[0:VW, :], acc1[0:VW, :])
                flush_tail()
                emit_tail(asb0, 0, NPAIR - 1, 3 * QB)
                emit_tail(asb1, 1, NPAIR - 1, 3 * QB)
                # final block's output projection
                for pair in range(NPAIR):
                    qt = 4 * 3 + pair
                    emit_oproj_unit(qt, 0)
                    emit_oproj_unit(qt, 1)

    nc.compile()
    return nc


def _get_prog():
    if _PROG[0] is None:
        _PROG[0] = _build()
    return _PROG[0]


def make_in_maps(query, key, value, Wq, Wk, Wv, Wo):
    import ml_dtypes

    bf = ml_dtypes.bfloat16
    scale = np.float32(1.0 / np.sqrt(D))
    Wq_s = (np.asarray(Wq, np.float32) * scale).astype(bf)
    Wk_s = np.asarray(Wk, np.float32).astype(bf)
    Wv_s = np.asarray(Wv, np.float32).astype(bf)
    Wo_s = np.asarray(Wo, np.float32).astype(bf)
    in_maps = []
    for b in range(B):
        xqT = np.ascontiguousarray(np.asarray(query[b], np.float32).T.astype(bf))
        xkT = np.ascontiguousarray(np.asarray(key[b], np.float32).T.astype(bf))
        xvT = np.ascontiguousarray(np.asarray(value[b], np.float32).T.astype(bf))
        for hh in range(2):
            sl = slice(hh * DH, (hh + 1) * DH)
            in_maps.append(
                {
                    "xq": xqT,
                    "xk": xkT,
                    "xv": xvT,
                    "wq": np.ascontiguousarray(Wq_s[:, sl]),
                    "wk": np.ascontiguousarray(Wk_s[:, sl]),
                    "wv": np.ascontiguousarray(Wv_s[:, sl]),
                    "wo": np.ascontiguousarray(Wo_s[sl, :]),
                }
            )
    return in_maps


def run(in_maps, trace=False, **kw):
    from concourse.bass_utils import run_bass_kernel_spmd

    nc = _get_prog()
    return run_bass_kernel_spmd(
        nc, in_maps, core_ids=list(range(NCORES)), trace=trace, **kw
    )


def kernel(query, key, value, Wq, Wk, Wv, Wo, bo):
    in_maps = make_in_maps(query, key, value, Wq, Wk, Wv, Wo)
    res = run(in_maps)
    bo = np.asarray(bo, np.float32)
    out = np.empty((B, S, D), np.float32)
    for b in range(B):
        out[b] = res.results[2 * b]["part"] + res.results[2 * b + 1]["part"] + bo
    return out


# revision 27
# speedup vs baseline: 1.0048x; 1.0048x over previous
"""Multi-head attention (B=4, S=2048, D=1024, H=16) on 8 trn2 NeuronCores.

Sharding: batch (4-way) x head-half (2-way).  Core c = 2*b + hh handles batch b
and heads hh*8 .. hh*8+7.  All matmuls run in bf16 (fp32 PSUM accumulation).

Per core:
  1. QT/KT projections in d-on-partitions layout, V in tokens-on-partitions
     layout with a ones-column per head (softmax denominator comes out of the
     attn@V matmul as row 64).
  2. Attention runs over 512-query blocks and head PAIRS (rows 0:64 / 64:128
     of one dim chunk): the two scores matmuls use disjoint PE row groups and
     overlap in the array.  Per (pair, kt): two scores matmuls, two exps (one
     on the scalar ACT pipe, one as a cubic+square-square polynomial custom op
     on the vector engine), two attn@V matmuls.  Fully double-buffered in
     8 PSUM banks (4 scores + 2 accumulators + 2 output-projection).
  3. Tail per head: denominator row replicated from PSUM with stream
     shuffles, fast reciprocal, multiply into bf16 outT.  Output projection
     for block qb-1 is interleaved into block qb's pair loop to fill PE slack.
     Host sums the two half-head partials and adds the bias.
"""

import sys

if "/opt/trn_rl_repo" not in sys.path:
    sys.path.insert(0, "/opt/trn_rl_repo")

import numpy as np

B, S, D = 4, 2048, 1024
H, HD = 16, 64
P = 128
DK = D // P            # 8 contraction chunks for the projections
NKT = S // P           # 16 token tiles
QB = 512
NQB = S // QB          # 4 q blocks
DH = 512               # head dims per core (8 heads)
NDC = DH // P          # 4 dout chunks per core
NHC = 8                # heads per core
NPAIR = NHC // 2       # head pairs per core
VW = HD + 1            # V columns per head incl. the ones column
NCORES = 8

_PROG = [None]

# exp(s) ~= (1 + s*(EA + s*(EB + s*EC)))**4, minimax on |s| <= 2.3 (scores
# measured in [-1.96, 2.08]); max rel err ~3e-3.
EA, EB, EC = 0.25036017, 0.03195562, 0.00254225


def _register_dve_exp():
    from concourse import dve_ops
    from concourse.dve_spec import Spec, Src0, One, C0, C1, C2, lower
    from concourse.dve_uop import DveOpSpec

    name = "EXP_P3SQ2_ANT"
    for op in dve_ops.OPS:
        if op.name == name:
            return op

    def ref(in0, in1, c0, c1, c2):
        x = np.asarray(in0, np.float32)
        p = (
            np.float32(1.0)
            + x * (np.float32(c0) + x * (np.float32(c1) + x * np.float32(c2)))
        ).astype(np.float32)
        q = (p * p).astype(np.float32)
        return (q * q).astype(np.float32)

    e = ((Src0 * C2 + C1) * Src0 + C0) * Src0
    p = e + One
    q = p * p
    spec = Spec(body=q * q, reference=ref)
    op = dve_ops.DveOp(name, spec, subdim=False, uops_sha={})
    dve_ops.OPS.append(op)
    dve_ops.CUSTOM_DVE_SPECS[name] = spec
    dve_ops._SUB_OPCODE_FOR_NAME[name] = dve_ops._CUSTOM_DVE_ROW_BASE + len(dve_ops.OPS) - 1
    for ver in ("v3", "v4"):
        s = DveOpSpec(
            name=name,
            opcode=dve_ops.get_dve_sub_opcode(name),
            uops=lower(spec, ver=ver),
            rd1_en=False,
        )
        op.uops_sha[ver] = s.sha(ver)
    return op


def _build():
    import concourse.mybir as mybir
    import concourse.tile as tile
    from concourse import bacc

    exp_op = _register_dve_exp()

    f32 = mybir.dt.float32
    bf16 = mybir.dt.bfloat16
    Exp = mybir.ActivationFunctionType.Exp

    nc = bacc.Bacc("TRN2", target_bir_lowering=False, debug=False)
    xq = nc.dram_tensor("xq", [D, S], bf16, kind="ExternalInput").ap()
    xk = nc.dram_tensor("xk", [D, S], bf16, kind="ExternalInput").ap()
    xv = nc.dram_tensor("xv", [D, S], bf16, kind="ExternalInput").ap()
    wq = nc.dram_tensor("wq", [D, DH], bf16, kind="ExternalInput").ap()
    wk = nc.dram_tensor("wk", [D, DH], bf16, kind="ExternalInput").ap()
    wv = nc.dram_tensor("wv", [D, DH], bf16, kind="ExternalInput").ap()
    wo = nc.dram_tensor("wo", [DH, D], bf16, kind="ExternalInput").ap()
    part = nc.dram_tensor("part", [S, D], f32, kind="ExternalOutput").ap()

    xq_v = xq.rearrange("(c p) s -> p c s", p=P)
    xk_v = xk.rearrange("(c p) s -> p c s", p=P)
    xv_v = xv.rearrange("(c p) s -> p c s", p=P)

    with tile.TileContext(nc) as tc:
        with tc.tile_pool(name="big", bufs=1) as big, tc.tile_pool(name="wp", bufs=2) as wp:
            QT = big.tile([P, NDC, S], bf16, tag="QT")
            KT = big.tile([P, NDC, S], bf16, tag="KT")
            V = big.tile([P, NKT, NHC * VW], bf16, tag="V")
            outT = big.tile([P, NDC, S], bf16, tag="outT")

            # ---- projections ------------------------------------------------
            with (
                tc.tile_pool(name="xc", bufs=12) as xc,
                tc.tile_pool(name="pp", bufs=4, space="PSUM") as pp,
            ):

                def proj_T(x_view, w_dram, out_t):
                    w_t = wp.tile([P, DK, DH], bf16, tag="w")
                    w_v = w_dram.rearrange("(c p) m -> p c m", p=P)
                    for dk in range(DK):
                        nc.sync.dma_start(w_t[:, dk], w_v[:, dk])
                    for qb in range(NQB):
                        xts = []
                        for dk in range(DK):
                            xt = xc.tile([P, QB], bf16, tag="xc")
                            nc.sync.dma_start(
                                xt[:], x_view[:, dk, qb * QB : (qb + 1) * QB]
                            )
                            xts.append(xt)
                        pts = [pp.tile([P, QB], f32, tag="pp", name=f"pp{i}") for i in range(NDC)]
                        for dk in range(DK):
                            for dc in range(NDC):
                                nc.tensor.matmul(
                                    pts[dc][:],
                                    w_t[:, dk, dc * P : (dc + 1) * P],
                                    xts[dk][:],
                                    start=(dk == 0),
                                    stop=(dk == DK - 1),
                                )
                        for dc in range(NDC):
                            dst = out_t[:, dc, qb * QB : (qb + 1) * QB]
                            if dc % 2 == 0:
                                nc.vector.tensor_copy(dst, pts[dc][:])
                            else:
                                nc.scalar.copy(dst, pts[dc][:])

                proj_T(xq_v, wq, QT)
                proj_T(xk_v, wk, KT)

                # V projection (tokens-on-partitions) + ones columns
                nc.vector.memset(V[:], 1.0)
                wv_t = wp.tile([P, DK, DH], bf16, tag="w")
                wv_v = wv.rearrange("(c p) m -> p c m", p=P)
                for dk in range(DK):
                    nc.sync.dma_start(wv_t[:, dk], wv_v[:, dk])
                for qb in range(NQB):
                    xts = []
                    for dk in range(DK):
                        xt = xc.tile([P, QB], bf16, tag="xc")
                        nc.sync.dma_start(
                            xt[:], xv_v[:, dk, qb * QB : (qb + 1) * QB]
                        )
                        xts.append(xt)
                    pts = [pp.tile([P, DH], f32, tag="pp", name=f"ppv{i}") for i in range(QB // P)]
                    for dk in range(DK):
                        for kt_in in range(QB // P):
                            nc.tensor.matmul(
                                pts[kt_in][:],
                                xts[dk][:, kt_in * P : (kt_in + 1) * P],
                                wv_t[:, dk, :],
                                start=(dk == 0),
                                stop=(dk == DK - 1),
                            )
                    for kt_in in range(QB // P):
                        kt = qb * (QB // P) + kt_in
                        nc.vector.tensor_copy(
                            V[:, kt].rearrange("p (h c) -> p h c", c=VW)[
                                :, :, 0:HD
                            ],
                            pts[kt_in][:].rearrange("p (h c) -> p h c", c=HD),
                        )

            # wo loads while attention runs (slot frees when wv_t is done)
            wo_t = wp.tile([P, NDC, D], bf16, tag="w")
            nc.sync.dma_start(wo_t[:], wo.rearrange("(c p) m -> p c m", p=P))

            # ---- attention + output projection, per 512-query block ---------
            with (
                tc.tile_pool(name="attn", bufs=6) as attnp,
                tc.tile_pool(name="tail", bufs=4) as tailp,
                tc.tile_pool(name="asbp", bufs=4) as asbp,
                tc.tile_pool(name="stage", bufs=2) as stage,
                tc.tile_pool(name="acc", bufs=2, space="PSUM") as accp,
                tc.tile_pool(name="sc", bufs=2, space="PSUM") as scp,
                tc.tile_pool(name="po", bufs=2, space="PSUM") as pop,
            ):

                def emit_oproj_unit(qt, do):
                    po = pop.tile([P, QB], f32, tag="po")
                    for dc in range(NDC):
                        nc.tensor.matmul(
                            po[:],
                            outT[:, dc, qt * P : (qt + 1) * P],
                            wo_t[:, dc, do * QB : (do + 1) * QB],
                            start=(dc == 0),
                            stop=(dc == NDC - 1),
                        )
                    st = stage.tile([P, QB], f32, tag="st")
                    if do == 0:
                        nc.vector.tensor_copy(st[:], po[:])
                    else:
                        nc.scalar.copy(st[:], po[:])
                    nc.sync.dma_start(
                        part[qt * P : (qt + 1) * P, do * QB : (do + 1) * QB],
                        st[:],
                    )

                def emit_score_pair_tile(kt, hc, c0):
                    # both heads of the pair into one [P, 2*QB] psum tile
                    # (bank-aligned halves); the two matmuls use disjoint PE
                    # row groups and overlap in the array
                    sct = scp.tile([P, 2 * QB], f32, tag="sc")
                    for hp in range(2):
                        r0 = 64 * hp
                        nc.tensor.matmul(
                            sct[:, hp * QB : (hp + 1) * QB],
                            KT[r0 : r0 + 64, hc, kt * P : (kt + 1) * P],
                            QT[r0 : r0 + 64, hc, c0 : c0 + QB],
                            start=True,
                            stop=True,
                        )
                    return sct

                def emit_exp(at_t, sct, on_act):
                    if on_act:
                        nc.scalar.activation(at_t[:], sct[:], Exp)
                    else:
                        nc.vector._custom_dve(
                            exp_op,
                            out=at_t[:],
                            in0=sct[:],
                            s0=EA,
                            s1=EB,
                            imm2=EC,
                        )

                def emit_attnv(acc, h, kt, at_t):
                    nc.tensor.matmul(
                        acc[0:VW, :],
                        V[:, kt, h * VW : (h + 1) * VW],
                        at_t[:],
                        start=(kt == 0),
                        stop=(kt == NKT - 1),
                    )

                def emit_tail(asb, hp, hc, c0):
                    # asb holds rows 0..64 of the accumulator (evacuated on
                    # the scalar engine): replicate the denominator row, fast
                    # reciprocal, divide into bf16 outT
                    bc = tailp.tile([64, QB], f32, tag="bc")
                    nc.vector.stream_shuffle(bc[0:32, :], asb[64:96, :], [0] * 32)
                    nc.vector.stream_shuffle(bc[32:64, :], asb[64:96, :], [0] * 32)
                    rec = tailp.tile([64, QB], f32, tag="rec")
                    nc.vector.reciprocal_approx_fast(rec[:], bc[:])
                    if hp == 0:
                        nc.vector.tensor_mul(
                            outT[0:64, hc, c0 : c0 + QB], asb[0:HD, :], rec[:]
                        )
                    else:
                        tmp = tailp.tile([64, QB], bf16, tag="tmp")
                        nc.vector.tensor_mul(tmp[:], asb[0:HD, :], rec[:])
                        nc.sync.dma_start(outT[64:128, hc, c0 : c0 + QB], tmp[:])

                # flat software pipeline over (qb, pair, kt): scores run one
                # step ahead, attn@V LAG steps behind, crossing pair
                # boundaries without drain/refill bubbles
                LAG = 2
                steps = [
                    (qb, pair, kt)
                    for qb in range(NQB)
                    for pair in range(NPAIR)
                    for kt in range(NKT)
                ]
                NS = len(steps)
                pending_tail = [None]

                def flush_tail():
                    if pending_tail[0] is not None:
                        a0, a1, phc, pc0 = pending_tail[0]
                        emit_tail(a0, 0, phc, pc0)
                        emit_tail(a1, 1, phc, pc0)
                        pending_tail[0] = None

                accs = {}
                scs = {}
                ats = {}

                def get_accs(qb, pair):
                    if (qb, pair) not in accs:
                        accs[(qb, pair)] = (
                            accp.tile([P, QB], f32, tag="acc", name="acc0"),
                            accp.tile([P, QB], f32, tag="acc", name="acc1"),
                        )
                    return accs[(qb, pair)]

                def emit_score_pair(g):
                    qb, pair, kt = steps[g]
                    scs[g] = emit_score_pair_tile(kt, pair, qb * QB)

                emit_score_pair(0)
                for g in range(NS):
                    qb, pair, kt = steps[g]
                    scq = scs.pop(g)
                    atq = attnp.tile([P, 2 * QB], bf16, tag="attn")
                    ats[g] = atq
                    emit_exp(atq, scq, on_act=(g % 2 == 0))
                    # attn@V (LAG steps behind) first, so the next scores'
                    # LDWEIGHTS can hide under these matmuls
                    if g >= LAG:
                        pqb, ppair, pkt = steps[g - LAG]
                        patq = ats.pop(g - LAG)
                        pa0, pa1 = get_accs(pqb, ppair)
                        emit_attnv(pa0, 2 * ppair, pkt, patq[:, 0:QB])
                        emit_attnv(pa1, 2 * ppair + 1, pkt, patq[:, QB : 2 * QB])
                        if pkt == NKT - 1:
                            # pair finished: evacuate accumulators on the
                            # scalar pipe so the acc banks recycle quickly
                            acc0, acc1 = accs.pop((pqb, ppair))
                            asb0 = asbp.tile([96, QB], f32, tag="asb", name="asb0")
                            asb1 = asbp.tile([96, QB], f32, tag="asb", name="asb1")
                            nc.scalar.copy(asb0[0:VW, :], acc0[0:VW, :])
                            nc.scalar.copy(asb1[0:VW, :], acc1[0:VW, :])
                            flush_tail()
                            pending_tail[0] = (asb0, asb1, ppair, pqb * QB)
                    if g + 1 < NS:
                        emit_score_pair(g + 1)
                    if kt == 6:
                        # previous pair's softmax tail, mid-loop so it fills
                        # vector-engine slack
                        flush_tail()
                    if qb > 0 and kt in (9, 13):
                        # previous block's output projection
                        qt = 4 * (qb - 1) + pair
                        emit_oproj_unit(qt, 0 if kt == 9 else 1)
                # drain the last LAG steps
                for g in range(NS - LAG, NS):
                    pqb, ppair, pkt = steps[g]
                    patq = ats.pop(g)
                    pa0, pa1 = get_accs(pqb, ppair)
                    emit_attnv(pa0, 2 * ppair, pkt, patq[:, 0:QB])
                    emit_attnv(pa1, 2 * ppair + 1, pkt, patq[:, QB : 2 * QB])
                acc0, acc1 = accs.pop((3, NPAIR - 1))
                asb0 = asbp.tile([96, QB], f32, tag="asb", name="asb0")
                asb1 = asbp.tile([96, QB], f32, tag="asb", name="asb1")
                nc.scalar.copy(asb0[0:VW, :], acc0[0:VW, :])
                nc.scalar.copy(asb1[0:VW, :], acc1[0:VW, :])
                flush_tail()
                emit_tail(asb0, 0, NPAIR - 1, 3 * QB)
                emit_tail(asb1, 1, NPAIR - 1, 3 * QB)
                # final block's output projection
                for pair in range(NPAIR):
                    qt = 4 * 3 + pair
                    emit_oproj_unit(qt, 0)
                    emit_oproj_unit(qt, 1)

    nc.compile()
    return nc


def _get_prog():
    if _PROG[0] is None:
        _PROG[0] = _build()
    return _PROG[0]


def make_in_maps(query, key, value, Wq, Wk, Wv, Wo):
    import ml_dtypes

    bf = ml_dtypes.bfloat16
    scale = np.float32(1.0 / np.sqrt(D))
    Wq_s = (np.asarray(Wq, np.float32) * scale).astype(bf)
    Wk_s = np.asarray(Wk, np.float32).astype(bf)
    Wv_s = np.asarray(Wv, np.float32).astype(bf)
    Wo_s = np.asarray(Wo, np.float32).astype(bf)
    in_maps = []
    for b in range(B):
        xqT = np.ascontiguousarray(np.asarray(query[b], np.float32).T.astype(bf))
        xkT = np.ascontiguousarray(np.asarray(key[b], np.float32).T.astype(bf))
        xvT = np.ascontiguousarray(np.asarray(value[b], np.float32).T.astype(bf))
        for hh in range(2):
            sl = slice(hh * DH, (hh + 1) * DH)
            in_maps.append(
                {
                    "xq": xqT,
                    "xk": xkT,
                    "xv": xvT,
                    "wq": np.ascontiguousarray(Wq_s[:, sl]),
                    "wk": np.ascontiguousarray(Wk_s[:, sl]),
                    "wv": np.ascontiguousarray(Wv_s[:, sl]),
                    "wo": np.ascontiguousarray(Wo_s[sl, :]),
                }
            )
    return in_maps


def run(in_maps, trace=False, **kw):
    from concourse.bass_utils import run_bass_kernel_spmd

    nc = _get_prog()
    return run_bass_kernel_spmd(
        nc, in_maps, core_ids=list(range(NCORES)), trace=trace, **kw
    )


def kernel(query, key, value, Wq, Wk, Wv, Wo, bo):
    in_maps = make_in_maps(query, key, value, Wq, Wk, Wv, Wo)
    res = run(in_maps)
    bo = np.asarray(bo, np.float32)
    out = np.empty((B, S, D), np.float32)
    for b in range(B):
        out[b] = res.results[2 * b]["part"] + res.results[2 * b + 1]["part"] + bo
    return out
